# revision 12
# baseline (speedup 1.0000x reference)
"""Multi-head attention kernel for Trainium2, SPMD over 8 NeuronCores.

Sharding: 2(batch) x 2(k-half) x 2(head-half). Each core holds one batch's
k/v slice of 4096 rows and computes K/V/Q projections + masked-softmax
attention for its 4 local heads. The only collectives are 2-rank
ReduceScatters (one ring step each, ~3x cheaper than the 4-rank variant)
that sum per-head attention numerators across the two k-halves; each core
then normalizes + projects its own q-half (256 rows) through its heads'
Wf rows, and the host sums the two head-half partials (same += gather the
4-way version used). No Q AllGather: each core projects Q for its own
heads locally.

Layout notes: all activations/weights/mask pre-transposed and pre-cast to
bf16 on the host; scores computed transposed ([k, q]) so the exp output is
directly the stationary operand of the AV matmul; multiplicative bf16 mask
after exp; softmax denominator rides as a 129th v-column through AV and
the ReduceScatter; no max-subtraction (scores are O(1)).

Engine schedule: scalar (ACT) runs only exp; vector owns all PSUM->SBUF
copies, mask multiplies, normalization and the f32 output accumulation.
Head 0's probs are precomputed during the V projection (ACT idle there);
the attention loop scores head s+1 while accumulating head s. Per-head RS
results are consumed two heads later (norm -> PE transpose -> per-head Wf
matmul -> f32 accum), so the PE never waits on a collective until the
final head's RS, whose post-work is ~3us. Startup DMAs are split into
512KB pieces across four queues (sync: xk stream, vector: wk, gpsimd: V
stream, scalar: wq/xq/wf/mask) so the first K-proj matmul starts ~1us in.
"""

import sys

if "/opt/trn_rl_repo" not in sys.path:
    sys.path.insert(0, "/opt/trn_rl_repo")

from contextlib import ExitStack

import ml_dtypes
import numpy as np

import concourse.bass as bass  # noqa: F401
import concourse.mybir as mybir
import concourse.tile as tile
from concourse import bacc
from concourse.masks import make_identity

B, QL, KL, D, H = 2, 512, 8192, 1024, 8
HD = D // H  # 128
NCORES = 8
PAIRS = [[0, 1], [2, 3], [4, 5], [6, 7]]  # k-half partners
KSH = KL // 2  # 4096 k rows per core
HL = 4  # local heads per core
SCALE = 1.0 / float(np.sqrt(HD))

F32 = mybir.dt.float32
BF16 = mybir.dt.bfloat16
P = 128
KC = KSH // P  # 32 k chunks of 128
QB = QL // P  # 4 q blocks
DB = D // P  # 8 d-in blocks
NCH = KSH // 512  # 8 streaming chunks of 512 k rows


def ensure_ntff_hook():
    """Provide antenv.axon_hooks (missing in this image) so trace=True works.

    Mirrors trn_agent_boot._ntff_profile_via_ctypes against the local
    libaxon_pjrt.so. No-op if the real module exists or the .so is absent.
    """
    try:
        import antenv.axon_hooks  # noqa: F401

        return
    except ImportError:
        pass
    import contextlib
    import ctypes
    import types

    mod = types.ModuleType("antenv.axon_hooks")
    holder = [None]
    mod.set_axon_ntff_profile_hook = lambda h: holder.__setitem__(0, h)
    mod.get_axon_ntff_profile_hook = lambda: holder[0]
    try:
        lib = ctypes.CDLL("/opt/axon/libaxon_pjrt.so")
        if hasattr(lib, "axon_start_nrt_profile"):
            lib.axon_start_nrt_profile.argtypes = [
                ctypes.POINTER(ctypes.c_int64),
                ctypes.c_size_t,
            ]
            lib.axon_start_nrt_profile.restype = ctypes.c_int64
            lib.axon_stop_nrt_profile.argtypes = [ctypes.c_char_p]
            lib.axon_stop_nrt_profile.restype = ctypes.c_int64

            @contextlib.contextmanager
            def _hook(output_dir, device_ids):
                import jax

                jax.devices()
                if device_ids:
                    ids = (ctypes.c_int64 * len(device_ids))(*device_ids)
                    rc = lib.axon_start_nrt_profile(ids, len(device_ids))
                else:
                    rc = lib.axon_start_nrt_profile(None, 0)
                if rc != 0:
                    raise RuntimeError(f"axon_start_nrt_profile rc={rc}")
                try:
                    yield
                finally:
                    n = lib.axon_stop_nrt_profile(str(output_dir).encode())
                    print(f"ntff profile: {n} file(s) -> {output_dir}")

            holder[0] = _hook
    except OSError:
        pass
    sys.modules["antenv.axon_hooks"] = mod
    try:
        import antenv

        antenv.axon_hooks = mod
    except ImportError:
        pass


def build_attention_kernel():
    nc = bacc.Bacc(
        "TRN2", target_bir_lowering=False, debug=False, num_devices=NCORES
    )

    xqT = nc.declare_dram_parameter("xqT", [D, QL], BF16, isOutput=False)
    xkT = nc.declare_dram_parameter("xkT", [D, KSH], BF16, isOutput=False)
    xvT = nc.declare_dram_parameter("xvT", [D, KSH], BF16, isOutput=False)
    mskT = nc.declare_dram_parameter("mskT", [KSH, QL], BF16, isOutput=False)
    wqT = nc.declare_dram_parameter("wqT", [D, HL * HD], BF16, isOutput=False)
    wkT = nc.declare_dram_parameter("wkT", [D, HL * HD], BF16, isOutput=False)
    wvT = nc.declare_dram_parameter("wvT", [D, HL * HD], BF16, isOutput=False)
    wfT = nc.declare_dram_parameter("wfT", [HL * HD, D], BF16, isOutput=False)
    out = nc.declare_dram_parameter("out", [2 * P, D], F32, isOutput=True)

    with tile.TileContext(nc) as tc, ExitStack() as ctx:
        # Persistent operand tiles (single-buffered, live for the kernel).
        persist = ctx.enter_context(tc.tile_pool(name="persist", bufs=1))
        kT = persist.tile([P, HL, KSH], BF16)  # [hd, head, krow]
        v_sb = persist.tile([P, KC, HL, HD + 1], BF16)  # [krow, kc, h, hd+1]
        mask_sb = persist.tile([P, KC, QL], BF16)  # [k, kc, q]
        qT = persist.tile([P, HL, QL], BF16)  # [hd, head, q]
        wv_sb = persist.tile([P, DB, HL * HD], BF16)
        wf_sb = persist.tile([P, HL, D], BF16)  # [hd, head, dout]
        sumT = persist.tile([P, HL, 2 * P], BF16)  # [hd, head, q-local]
        rsn = [
            persist.tile([P, 2, HD + 1], BF16, name=f"rsn{h}") for h in range(HL)
        ]
        rden = persist.tile([P, HL, 2], F32)
        out_acc = persist.tile([P, 2, 2, 512], F32)  # [q, qb-own, n, dout-half]

        # 32KB alias block: wk/wq/xq early, head-0 precomputed probs late.
        # (wk dies at K-proj end, wq/xq after the Q projection; pp0's first
        # write happens during the V projection, strictly later.)
        ablk = persist.tile([P, KC, QL], BF16, name="ablk")
        wk_sb = ablk[:, 0:DB, :]  # [P, 8, 512]
        wq_sb = ablk[:, DB : 2 * DB, :]
        xq_sb = ablk[:, 2 * DB : 3 * DB, :]
        pp0 = ablk  # [P, KC, QL] head-0 probs, precomputed

        loads = ctx.enter_context(tc.tile_pool(name="loads", bufs=2))
        probs_pool = ctx.enter_context(tc.tile_pool(name="probs", bufs=7))
        nums = ctx.enter_context(tc.tile_pool(name="nums", bufs=2))
        small = ctx.enter_context(tc.tile_pool(name="small", bufs=4))
        dram = ctx.enter_context(tc.tile_pool(name="dram", bufs=1, space="DRAM"))

        # rs_in[h]: [2 shares, 2 qb, 128, 129]; share r -> group rank r
        # (rank = k-half), i.e. q rows r*256..r*256+255 stay with rank r.
        rs_in = [
            dram.tile([2, 2, P, HD + 1], BF16, name=f"rs_in{h}")
            for h in range(HL)
        ]
        rs_out = [
            dram.tile([2, P, HD + 1], BF16, name=f"rs_out{h}") for h in range(HL)
        ]
        # tiny warmup collective: pays the first-call ncfw/CC staging cost
        # during the DMA-load phase instead of inside the attention loop.
        warm_in = dram.tile([2, P, 2], BF16, name="warm_in")
        warm_out = dram.tile([P, 2], BF16, name="warm_out")

        consts = ctx.enter_context(tc.tile_pool(name="consts", bufs=1))
        ident = consts.tile([P, P], BF16)
        make_identity(nc, ident)

        # One PSUM pool, 8 banks: mm 2x2 + av 4x1.
        psum = ctx.enter_context(tc.tile_pool(name="psum", bufs=1, space="PSUM"))

        def mm_tile(name, dtype=F32):
            return psum.tile([P, 2, 512], dtype, tag="mm", bufs=2, name=name)

        def av_tile(name, cols=HD + 1):
            return psum.tile([P, cols], F32, tag="av", bufs=4, name=name)

        # --- DMA loads, split across three queues so the first K-proj
        # matmul can start ~1.5us in.
        #   sync:   xk stream (critical path; chunks 0-1 split finer)
        #   scalar: wk first (parallel with xk), then wq, xq, wf
        #   gpsimd: wv, mask, then xv stream
        def load_pair(eng, dst, src, lo, hi):
            eng.dma_start(
                out=dst[:, lo // P : hi // P, :],
                in_=src[lo:hi, :].rearrange("(a p) d -> p a d", p=P),
            )

        def load_xchunk(eng, dst, src, c, half=None):
            lo, hi = (0, D) if half is None else (half * 512, (half + 1) * 512)
            eng.dma_start(
                out=dst[:, lo // P : hi // P, :],
                in_=src[lo:hi, c * 512 : (c + 1) * 512].rearrange(
                    "(a p) k -> p a k", p=P
                ),
            )

        load_pair(nc.scalar, wk_sb, wkT, 0, 512)
        load_pair(nc.scalar, wk_sb, wkT, 512, 1024)
        xkc0 = loads.tile([P, DB, 512], BF16, tag="ld", name="xkc0")
        for half in range(2):
            load_xchunk(nc.sync, xkc0, xkT, 0, half)
        xkc1 = loads.tile([P, DB, 512], BF16, tag="ld", name="xkc1")
        for half in range(2):
            load_xchunk(nc.sync, xkc1, xkT, 1, half)

        load_pair(nc.scalar, wq_sb, wqT, 0, 1024)
        load_pair(nc.scalar, xq_sb, xqT, 0, 1024)
        nc.scalar.dma_start(
            out=wf_sb, in_=wfT.rearrange("(i p) d -> p i d", p=P)
        )

        load_pair(nc.gpsimd, wv_sb, wvT, 0, 1024)
        for i in range(2):
            nc.gpsimd.dma_start(
                out=mask_sb[:, i * 16 : (i + 1) * 16, :],
                in_=mskT[i * 2048 : (i + 1) * 2048, :].rearrange(
                    "(a p) q -> p a q", p=P
                ),
            )
        # warmup collective (result unused)
        nc.vector.memset(sumT[:, 0:2, 0:2], 0.0)
        nc.sync.dma_start(
            out=warm_in.rearrange("s p c -> p s c"), in_=sumT[:, 0:2, 0:2]
        )
        nc.gpsimd.collective_compute(
            "ReduceScatter",
            mybir.AluOpType.add,
            replica_groups=PAIRS,
            ins=[warm_in.opt()],
            outs=[warm_out.opt()],
        )

        # --- K projection: 8 chunks of 512 k rows; 2 head-pairs each.
        def k_proj_chunk(c, xkc):
            for hp in range(2):
                pk = mm_tile(f"pk_{c}_{hp}")
                for i in range(2):
                    for a in range(DB):
                        nc.tensor.matmul(
                            pk[:, i, :],
                            wk_sb[:, a, hp * 256 + i * HD : hp * 256 + (i + 1) * HD],
                            xkc[:, a, :],
                            start=(a == 0),
                            stop=(a == DB - 1),
                        )
                nc.vector.tensor_copy(
                    out=kT[:, 2 * hp : 2 * hp + 2, c * 512 : (c + 1) * 512],
                    in_=pk[:],
                )

        k_proj_chunk(0, xkc0)
        xkc2 = loads.tile([P, DB, 512], BF16, tag="ld", name="xkc2")
        load_xchunk(nc.sync, xkc2, xkT, 2)
        k_proj_chunk(1, xkc1)
        xkc3 = loads.tile([P, DB, 512], BF16, tag="ld", name="xkc3")
        load_xchunk(nc.sync, xkc3, xkT, 3)
        k_proj_chunk(2, xkc2)

        # --- Q projection for this core's 4 heads (local; no AllGather).
        # Placed here so its wq/xq DMAs (behind wk on scalar) have landed.
        for hp in range(2):
            pq = mm_tile(f"pq_{hp}")
            for i in range(2):
                for a in range(DB):
                    nc.tensor.matmul(
                        pq[:, i, :],
                        wq_sb[:, a, hp * 256 + i * HD : hp * 256 + (i + 1) * HD],
                        xq_sb[:, a, :],
                        start=(a == 0),
                        stop=(a == DB - 1),
                    )
            nc.vector.tensor_copy(out=qT[:, 2 * hp : 2 * hp + 2, :], in_=pq[:])

        nxt = xkc3
        for c in range(3, NCH):
            if c < NCH - 1:
                nxtc = loads.tile([P, DB, 512], BF16, tag="ld", name=f"xkc{c + 1}")
                load_xchunk(nc.sync, nxtc, xkT, c + 1)
            k_proj_chunk(c, nxt)
            nxt = nxtc if c < NCH - 1 else None

        # --- V projection (xvT streamed); head-0 probs precomputed alongside
        # (ACT is otherwise idle here). One pre_probs per (c, mkl).
        def pre_probs(kc):
            ps = av_tile(f"pps_{kc}", 512)
            nc.tensor.matmul(
                ps[:],
                kT[:, 0, kc * P : (kc + 1) * P],
                qT[:, 0, :],
                start=True,
                stop=True,
            )
            nc.scalar.activation(
                pp0[:, kc, :], ps[:], mybir.ActivationFunctionType.Exp, scale=SCALE
            )
            nc.vector.tensor_mul(
                pp0[:, kc, :], pp0[:, kc, :], mask_sb[:, kc, :]
            )

        xvc_next = loads.tile([P, DB, 512], BF16, tag="ld", name="xvc0")
        load_xchunk(nc.gpsimd, xvc_next, xvT, 0)
        for c in range(NCH):
            xvc = xvc_next
            if c < NCH - 1:
                xvc_next = loads.tile(
                    [P, DB, 512], BF16, tag="ld", name=f"xvc{c + 1}"
                )
                load_xchunk(nc.gpsimd, xvc_next, xvT, c + 1)
            for mkl in range(4):
                mk = c * 4 + mkl
                pv = av_tile(f"pv_{mk}", 512)
                for a in range(DB):
                    nc.tensor.matmul(
                        pv[:],
                        xvc[:, a, mkl * P : (mkl + 1) * P],
                        wv_sb[:, a, :],
                        start=(a == 0),
                        stop=(a == DB - 1),
                    )
                nc.vector.tensor_copy(
                    out=v_sb[:, mk, :, 0:HD],
                    in_=pv[:].rearrange("p (b c) -> p b c", b=HL),
                )
                pre_probs(mk)
        nc.vector.memset(v_sb[:, :, :, HD], 1.0)

        # --- per-head numerator ReduceScatter (2-rank, k-half partners).
        def rs_fire(h):
            nc.gpsimd.collective_compute(
                "ReduceScatter",
                mybir.AluOpType.add,
                replica_groups=PAIRS,
                ins=[rs_in[h].opt()],
                outs=[rs_out[h].opt()],
            )

        def rsn_load(h):
            nc.sync.dma_start(
                out=rsn[h][:], in_=rs_out[h].rearrange("b p c -> p b c")
            )

        # normalize + transpose + project piece h, accumulating into out_acc.
        def process_piece(h):
            nc.vector.tensor_copy(out=rden[:, h, :], in_=rsn[h][:, :, HD])
            # guard fully-masked rows (reference wipes them to 0): 0/eps -> 0
            nc.vector.tensor_scalar_max(rden[:, h, :], rden[:, h, :], 1e-30)
            nc.vector.reciprocal(rden[:, h, :], rden[:, h, :])
            snorms = []
            for b in range(2):
                snorm = small.tile([P, HD], BF16, tag="snorm", name=f"sn_{h}_{b}")
                nc.vector.tensor_scalar_mul(
                    snorm[:],
                    rsn[h][:, b, 0:HD],
                    rden[:, h, b : b + 1],
                )
                snorms.append(snorm)
            pst = mm_tile(f"st_{h}", BF16)
            for b in range(2):
                nc.tensor.transpose(
                    pst[:, 0, b * P : (b + 1) * P], snorms[b][:], ident
                )
            nc.vector.tensor_copy(out=sumT[:, h, :], in_=pst[:, 0, 0 : 2 * P])
            for b in range(2):
                po = mm_tile(f"po_{h}_{b}")
                for n in range(2):
                    nc.tensor.matmul(
                        po[:, n, :],
                        sumT[:, h, b * P : (b + 1) * P],
                        wf_sb[:, h, n * 512 : (n + 1) * 512],
                        start=True,
                        stop=True,
                    )
                if h == 0:
                    nc.vector.tensor_copy(out=out_acc[:, b], in_=po[:])
                else:
                    nc.vector.tensor_add(out_acc[:, b], out_acc[:, b], po[:])

        # --- attention pipeline: score head s+1 while accumulating head s;
        # fire a 2-rank RS after every head; pieces 0/1 are consumed two
        # heads later (hidden), pieces 2/3 in the short tail.
        for s in range(HL):
            avs = [av_tile(f"av_{s}_{qb}") for qb in range(QB)]
            prs = []
            for j in range(KC // 2):
                if s < HL - 1:
                    hn = s + 1
                    ps = mm_tile(f"ps_{hn}_{j}")
                    for half in range(2):
                        kc = j * 2 + half
                        nc.tensor.matmul(
                            ps[:, half, :],
                            kT[:, hn, kc * P : (kc + 1) * P],
                            qT[:, hn, :],
                            start=True,
                            stop=True,
                        )
                    pr = probs_pool.tile(
                        [P, 2, 512], BF16, tag="probs", name=f"pr_{hn}_{j}"
                    )
                    nc.scalar.activation(
                        pr[:], ps[:], mybir.ActivationFunctionType.Exp, scale=SCALE
                    )
                    # split mask multiplies across DVE and GpSimd so DVE
                    # keeps slack for the PSUM copies at head boundaries
                    meng = nc.vector if j % 2 == 0 else nc.gpsimd
                    meng.tensor_mul(
                        pr[:], pr[:], mask_sb[:, j * 2 : j * 2 + 2, :]
                    )
                    prs.append(pr)
                # AV for head s, k-chunks 2j, 2j+1
                for half in range(2):
                    kc = j * 2 + half
                    for qb in range(QB):
                        if s == 0:
                            lhs = pp0[:, kc, qb * P : (qb + 1) * P]
                        else:
                            lhs = cur_prs[j][:, half, qb * P : (qb + 1) * P]
                        nc.tensor.matmul(
                            avs[qb][:],
                            lhs,
                            v_sb[:, kc, s, :],
                            start=(kc == 0),
                            stop=(kc == KC - 1),
                        )
            cur_prs = prs
            num = nums.tile([P, QB, HD + 1], BF16, tag="num", name=f"num_{s}")
            for qb in range(QB):
                nc.vector.tensor_copy(out=num[:, qb, :], in_=avs[qb][:])
            nc.sync.dma_start(
                out=rs_in[s].rearrange("a b p c -> p (a b) c"),
                in_=num[:],
            )
            rs_fire(s)
            # head h's RS (fired after head h) has ~2 heads of slack;
            # load + normalize + project it two heads later.
            if s >= 2:
                h = s - 2
                rsn_load(h)
                process_piece(h)

        for h in (HL - 2, HL - 1):
            rsn_load(h)
            process_piece(h)

        # --- store the [256, 1024] f32 partial (host sums head-halves).
        engs = [nc.sync, nc.scalar]
        for b in range(2):
            engs[b].dma_start(
                out=out[b * P : (b + 1) * P, :],
                in_=out_acc[:, b].rearrange("p n d -> p (n d)"),
            )

    nc.compile()
    return nc


_NC_CACHE = None


def _get_nc():
    global _NC_CACHE
    if _NC_CACHE is None:
        _NC_CACHE = build_attention_kernel()
    return _NC_CACHE


def make_in_maps(inputs):
    BF = ml_dtypes.bfloat16
    inputs = {k: np.asarray(v) for k, v in inputs.items()}
    WqT = np.asarray(inputs["Wq"]).T.astype(BF)  # [din, dout]
    WkT = np.asarray(inputs["Wk"]).T.astype(BF)
    WvT = np.asarray(inputs["Wv"]).T.astype(BF)
    WfT = np.asarray(inputs["Wf"]).T.astype(BF)  # [dsum, dout]
    xqTs = [
        np.ascontiguousarray(inputs["inputs_q"][b].T.astype(BF)) for b in range(B)
    ]
    in_maps = []
    for c in range(NCORES):
        b, hh, kh = c // 4, (c % 4) // 2, c % 2
        sl = slice(kh * KSH, (kh + 1) * KSH)
        hs = slice(hh * HL * HD, (hh + 1) * HL * HD)
        in_maps.append(
            {
                "xqT": xqTs[b],
                "xkT": np.ascontiguousarray(inputs["inputs_k"][b, sl].T.astype(BF)),
                "xvT": np.ascontiguousarray(inputs["inputs_v"][b, sl].T.astype(BF)),
                "mskT": np.ascontiguousarray(
                    inputs["attention_mask"][b, :, sl].T.astype(BF)
                ),
                "wqT": np.ascontiguousarray(WqT[:, hs]),
                "wkT": np.ascontiguousarray(WkT[:, hs]),
                "wvT": np.ascontiguousarray(WvT[:, hs]),
                "wfT": np.ascontiguousarray(WfT[hs, :]),
            }
        )
    return in_maps


def gather_out(results):
    out = np.zeros((B, QL, D), np.float32)
    for c in range(NCORES):
        b, kh = c // 4, c % 2
        r0 = kh * 256
        out[b, r0 : r0 + 256] += results[c]["out"]
    return out


def kernel(**inputs) -> np.ndarray:
    ensure_ntff_hook()  # defensive: BASS_TRACE=1 in env would need the shim
    from concourse.bass_utils import run_bass_kernel_spmd

    nc = _get_nc()
    in_maps = make_in_maps(inputs)
    res = run_bass_kernel_spmd(nc, in_maps, list(range(NCORES)))
    return gather_out(res.results)


# revision 15
# speedup vs baseline: 1.0784x; 1.0784x over previous
"""Multi-head attention kernel for Trainium2, SPMD over 8 NeuronCores.

Sharding: 2(batch) x 2(k-half) x 2(head-half). Each core holds one batch's
k/v slice of 4096 rows and computes K/V/Q projections + masked-softmax
attention for its 4 local heads. The only collectives are 2-rank
ReduceScatters (one ring step each, ~3x cheaper than the 4-rank variant)
that sum per-head attention numerators across the two k-halves; each core
then normalizes + projects its own q-half (256 rows) through its heads'
Wf rows, and the host sums the two head-half partials (same += gather the
4-way version used). No Q AllGather: each core projects Q for its own
heads locally.

Layout notes: all activations/weights/mask pre-transposed and pre-cast to
bf16 on the host; scores computed transposed ([k, q]) so the exp output is
directly the stationary operand of the AV matmul; multiplicative bf16 mask
after exp; softmax denominator rides as a 129th v-column through AV and
the ReduceScatter; no max-subtraction (scores are O(1)).

Engine schedule: scalar (ACT) runs only exp; vector owns all PSUM->SBUF
copies, mask multiplies, normalization and the f32 output accumulation.
Head 0's probs are precomputed during the V projection (ACT idle there);
the attention loop scores head s+1 while accumulating head s. Per-head RS
results are consumed two heads later (norm -> PE transpose -> per-head Wf
matmul -> f32 accum), so the PE never waits on a collective until the
final head's RS, whose post-work is ~3us. Startup DMAs are split into
512KB pieces across four queues (sync: xk stream, vector: wk, gpsimd: V
stream, scalar: wq/xq/wf/mask) so the first K-proj matmul starts ~1us in.
"""

import sys

if "/opt/trn_rl_repo" not in sys.path:
    sys.path.insert(0, "/opt/trn_rl_repo")

from contextlib import ExitStack

import ml_dtypes
import numpy as np

import concourse.bass as bass  # noqa: F401
import concourse.mybir as mybir
import concourse.tile as tile
from concourse import bacc
from concourse.masks import make_identity

B, QL, KL, D, H = 2, 512, 8192, 1024, 8
HD = D // H  # 128
NCORES = 8
PAIRS = [[0, 1], [2, 3], [4, 5], [6, 7]]  # k-half partners
KSH = KL // 2  # 4096 k rows per core
HL = 4  # local heads per core
SCALE = 1.0 / float(np.sqrt(HD))

F32 = mybir.dt.float32
BF16 = mybir.dt.bfloat16
P = 128
KC = KSH // P  # 32 k chunks of 128
QB = QL // P  # 4 q blocks
DB = D // P  # 8 d-in blocks
NCH = KSH // 512  # 8 streaming chunks of 512 k rows


def ensure_ntff_hook():
    """Provide antenv.axon_hooks (missing in this image) so trace=True works.

    Mirrors trn_agent_boot._ntff_profile_via_ctypes against the local
    libaxon_pjrt.so. No-op if the real module exists or the .so is absent.
    """
    try:
        import antenv.axon_hooks  # noqa: F401

        return
    except ImportError:
        pass
    import contextlib
    import ctypes
    import types

    mod = types.ModuleType("antenv.axon_hooks")
    holder = [None]
    mod.set_axon_ntff_profile_hook = lambda h: holder.__setitem__(0, h)
    mod.get_axon_ntff_profile_hook = lambda: holder[0]
    try:
        lib = ctypes.CDLL("/opt/axon/libaxon_pjrt.so")
        if hasattr(lib, "axon_start_nrt_profile"):
            lib.axon_start_nrt_profile.argtypes = [
                ctypes.POINTER(ctypes.c_int64),
                ctypes.c_size_t,
            ]
            lib.axon_start_nrt_profile.restype = ctypes.c_int64
            lib.axon_stop_nrt_profile.argtypes = [ctypes.c_char_p]
            lib.axon_stop_nrt_profile.restype = ctypes.c_int64

            @contextlib.contextmanager
            def _hook(output_dir, device_ids):
                import jax

                jax.devices()
                if device_ids:
                    ids = (ctypes.c_int64 * len(device_ids))(*device_ids)
                    rc = lib.axon_start_nrt_profile(ids, len(device_ids))
                else:
                    rc = lib.axon_start_nrt_profile(None, 0)
                if rc != 0:
                    raise RuntimeError(f"axon_start_nrt_profile rc={rc}")
                try:
                    yield
                finally:
                    n = lib.axon_stop_nrt_profile(str(output_dir).encode())
                    print(f"ntff profile: {n} file(s) -> {output_dir}")

            holder[0] = _hook
    except OSError:
        pass
    sys.modules["antenv.axon_hooks"] = mod
    try:
        import antenv

        antenv.axon_hooks = mod
    except ImportError:
        pass


def build_attention_kernel():
    nc = bacc.Bacc(
        "TRN2", target_bir_lowering=False, debug=False, num_devices=NCORES
    )

    xqT = nc.declare_dram_parameter("xqT", [D, QL], BF16, isOutput=False)
    xkT = nc.declare_dram_parameter("xkT", [D, KSH], BF16, isOutput=False)
    xvT = nc.declare_dram_parameter("xvT", [D, KSH], BF16, isOutput=False)
    mskT = nc.declare_dram_parameter("mskT", [KSH, QL], BF16, isOutput=False)
    wqT = nc.declare_dram_parameter("wqT", [D, HL * HD], BF16, isOutput=False)
    wkT = nc.declare_dram_parameter("wkT", [D, HL * HD], BF16, isOutput=False)
    wvT = nc.declare_dram_parameter("wvT", [D, HL * HD], BF16, isOutput=False)
    wfT = nc.declare_dram_parameter("wfT", [HL * HD, D], BF16, isOutput=False)
    out = nc.declare_dram_parameter("out", [2 * P, D], F32, isOutput=True)

    with tile.TileContext(nc) as tc, ExitStack() as ctx:
        # Persistent operand tiles (single-buffered, live for the kernel).
        persist = ctx.enter_context(tc.tile_pool(name="persist", bufs=1))
        kT = persist.tile([P, HL, KSH], BF16)  # [hd, head, krow]
        v_sb = persist.tile([P, KC, HL, HD + 1], BF16)  # [krow, kc, h, hd+1]
        mask_sb = persist.tile([P, KC, QL], BF16)  # [k, kc, q]
        qT = persist.tile([P, HL, QL], BF16)  # [hd, head, q]
        wv_sb = persist.tile([P, DB, HL * HD], BF16)
        wf_sb = persist.tile([P, HL, D], BF16)  # [hd, head, dout]
        sumT = persist.tile([P, HL, 2 * P], BF16)  # [hd, head, q-local]
        rsn = [
            persist.tile([P, 2, HD + 1], BF16, name=f"rsn{h}") for h in range(HL)
        ]
        rden = persist.tile([P, HL, 2], F32)
        out_acc = persist.tile([P, 2, 2, 512], F32)  # [q, qb-own, n, dout-half]

        # 32KB alias block: wk/wq/xq early, head-0 precomputed probs late.
        # (wk dies at K-proj end, wq/xq after the Q projection; pp0's first
        # write happens during the V projection, strictly later.)
        ablk = persist.tile([P, KC, QL], BF16, name="ablk")
        wk_sb = ablk[:, 0:DB, :]  # [P, 8, 512]
        wq_sb = ablk[:, DB : 2 * DB, :]
        xq_sb = ablk[:, 2 * DB : 3 * DB, :]
        pp0 = ablk  # [P, KC, QL] head-0 probs, precomputed

        loads = ctx.enter_context(tc.tile_pool(name="loads", bufs=2))
        probs_pool = ctx.enter_context(tc.tile_pool(name="probs", bufs=7))
        nums = ctx.enter_context(tc.tile_pool(name="nums", bufs=2))
        small = ctx.enter_context(tc.tile_pool(name="small", bufs=4))
        dram = ctx.enter_context(tc.tile_pool(name="dram", bufs=1, space="DRAM"))

        # rs_in[h]: [2 shares, 2 qb, 128, 129]; share r -> group rank r
        # (rank = k-half), i.e. q rows r*256..r*256+255 stay with rank r.
        rs_in = [
            dram.tile([2, 2, P, HD + 1], BF16, name=f"rs_in{h}")
            for h in range(HL)
        ]
        rs_out = [
            dram.tile([2, P, HD + 1], BF16, name=f"rs_out{h}") for h in range(HL)
        ]
        # tiny warmup collective: pays the first-call ncfw/CC staging cost
        # during the DMA-load phase instead of inside the attention loop.
        warm_in = dram.tile([2, P, 2], BF16, name="warm_in")
        warm_out = dram.tile([P, 2], BF16, name="warm_out")

        consts = ctx.enter_context(tc.tile_pool(name="consts", bufs=1))
        ident = consts.tile([P, P], BF16)
        make_identity(nc, ident)

        # One PSUM pool, 8 banks: mm 2x2 + av 4x1.
        psum = ctx.enter_context(tc.tile_pool(name="psum", bufs=1, space="PSUM"))

        def mm_tile(name, dtype=F32):
            return psum.tile([P, 2, 512], dtype, tag="mm", bufs=2, name=name)

        def av_tile(name, cols=HD + 1):
            return psum.tile([P, cols], F32, tag="av", bufs=4, name=name)

        # --- DMA loads, split across three queues so the first K-proj
        # matmul can start ~1.5us in.
        #   sync:   xk stream (critical path; chunks 0-1 split finer)
        #   scalar: wk first (parallel with xk), then wq, xq, wf
        #   gpsimd: wv, mask, then xv stream
        def load_pair(eng, dst, src, lo, hi):
            eng.dma_start(
                out=dst[:, lo // P : hi // P, :],
                in_=src[lo:hi, :].rearrange("(a p) d -> p a d", p=P),
            )

        def load_xchunk(eng, dst, src, c, half=None):
            lo, hi = (0, D) if half is None else (half * 512, (half + 1) * 512)
            eng.dma_start(
                out=dst[:, lo // P : hi // P, :],
                in_=src[lo:hi, c * 512 : (c + 1) * 512].rearrange(
                    "(a p) k -> p a k", p=P
                ),
            )

        load_pair(nc.scalar, wk_sb, wkT, 0, 512)
        load_pair(nc.scalar, wk_sb, wkT, 512, 1024)
        xkc0 = loads.tile([P, DB, 512], BF16, tag="ld", name="xkc0")
        for half in range(2):
            load_xchunk(nc.sync, xkc0, xkT, 0, half)
        xkc1 = loads.tile([P, DB, 512], BF16, tag="ld", name="xkc1")
        for half in range(2):
            load_xchunk(nc.sync, xkc1, xkT, 1, half)

        load_pair(nc.scalar, wq_sb, wqT, 0, 1024)
        load_pair(nc.scalar, xq_sb, xqT, 0, 1024)
        nc.scalar.dma_start(
            out=wf_sb, in_=wfT.rearrange("(i p) d -> p i d", p=P)
        )

        load_pair(nc.gpsimd, wv_sb, wvT, 0, 1024)
        for i in range(2):
            nc.gpsimd.dma_start(
                out=mask_sb[:, i * 16 : (i + 1) * 16, :],
                in_=mskT[i * 2048 : (i + 1) * 2048, :].rearrange(
                    "(a p) q -> p a q", p=P
                ),
            )
        # warmup collective staging DMA (collective fired after the V DMA
        # stream is fully issued; the trigger blocks the gpsimd engine).
        nc.vector.memset(sumT[:, 0:2, 0:2], 0.0)
        nc.sync.dma_start(
            out=warm_in.rearrange("s p c -> p s c"), in_=sumT[:, 0:2, 0:2]
        )

        # --- K projection: 8 chunks of 512 k rows; 2 head-pairs each.
        def k_proj_chunk(c, xkc):
            for hp in range(2):
                pk = mm_tile(f"pk_{c}_{hp}")
                for i in range(2):
                    for a in range(DB):
                        nc.tensor.matmul(
                            pk[:, i, :],
                            wk_sb[:, a, hp * 256 + i * HD : hp * 256 + (i + 1) * HD],
                            xkc[:, a, :],
                            start=(a == 0),
                            stop=(a == DB - 1),
                        )
                nc.vector.tensor_copy(
                    out=kT[:, 2 * hp : 2 * hp + 2, c * 512 : (c + 1) * 512],
                    in_=pk[:],
                )

        k_proj_chunk(0, xkc0)
        xkc2 = loads.tile([P, DB, 512], BF16, tag="ld", name="xkc2")
        load_xchunk(nc.sync, xkc2, xkT, 2)
        k_proj_chunk(1, xkc1)
        xkc3 = loads.tile([P, DB, 512], BF16, tag="ld", name="xkc3")
        load_xchunk(nc.sync, xkc3, xkT, 3)
        k_proj_chunk(2, xkc2)

        # --- Q projection for this core's 4 heads (local; no AllGather).
        # Placed here so its wq/xq DMAs (behind wk on scalar) have landed.
        for hp in range(2):
            pq = mm_tile(f"pq_{hp}")
            for i in range(2):
                for a in range(DB):
                    nc.tensor.matmul(
                        pq[:, i, :],
                        wq_sb[:, a, hp * 256 + i * HD : hp * 256 + (i + 1) * HD],
                        xq_sb[:, a, :],
                        start=(a == 0),
                        stop=(a == DB - 1),
                    )
            nc.vector.tensor_copy(out=qT[:, 2 * hp : 2 * hp + 2, :], in_=pq[:])

        nxt = xkc3
        for c in range(3, NCH):
            if c < NCH - 1:
                nxtc = loads.tile([P, DB, 512], BF16, tag="ld", name=f"xkc{c + 1}")
                load_xchunk(nc.sync, nxtc, xkT, c + 1)
            k_proj_chunk(c, nxt)
            nxt = nxtc if c < NCH - 1 else None

        # --- V projection (xvT streamed); head-0 probs precomputed alongside
        # (ACT is otherwise idle here). One pre_probs per (c, mkl).
        def pre_probs(kc):
            ps = av_tile(f"pps_{kc}", 512)
            nc.tensor.matmul(
                ps[:],
                kT[:, 0, kc * P : (kc + 1) * P],
                qT[:, 0, :],
                start=True,
                stop=True,
            )
            nc.scalar.activation(
                pp0[:, kc, :], ps[:], mybir.ActivationFunctionType.Exp, scale=SCALE
            )
            nc.vector.tensor_mul(
                pp0[:, kc, :], pp0[:, kc, :], mask_sb[:, kc, :]
            )

        xvc_next = loads.tile([P, DB, 512], BF16, tag="ld", name="xvc0")
        load_xchunk(nc.gpsimd, xvc_next, xvT, 0)
        for c in range(NCH):
            xvc = xvc_next
            if c < NCH - 1:
                xvc_next = loads.tile(
                    [P, DB, 512], BF16, tag="ld", name=f"xvc{c + 1}"
                )
                load_xchunk(nc.gpsimd, xvc_next, xvT, c + 1)
            if c == NCH - 1:
                # warmup collective (result unused): absorbs the first-call
                # CC staging latency while the PE finishes the V projection.
                nc.gpsimd.collective_compute(
                    "ReduceScatter",
                    mybir.AluOpType.add,
                    replica_groups=PAIRS,
                    ins=[warm_in.opt()],
                    outs=[warm_out.opt()],
                )
            for mkl in range(4):
                mk = c * 4 + mkl
                pv = av_tile(f"pv_{mk}", 512)
                for a in range(DB):
                    nc.tensor.matmul(
                        pv[:],
                        xvc[:, a, mkl * P : (mkl + 1) * P],
                        wv_sb[:, a, :],
                        start=(a == 0),
                        stop=(a == DB - 1),
                    )
                nc.vector.tensor_copy(
                    out=v_sb[:, mk, :, 0:HD],
                    in_=pv[:].rearrange("p (b c) -> p b c", b=HL),
                )
                pre_probs(mk)
        nc.vector.memset(v_sb[:, :, :, HD], 1.0)

        # --- per-head numerator ReduceScatter (2-rank, k-half partners).
        def rs_fire(h):
            nc.gpsimd.collective_compute(
                "ReduceScatter",
                mybir.AluOpType.add,
                replica_groups=PAIRS,
                ins=[rs_in[h].opt()],
                outs=[rs_out[h].opt()],
            )

        def rsn_load(h):
            nc.sync.dma_start(
                out=rsn[h][:], in_=rs_out[h].rearrange("b p c -> p b c")
            )

        # normalize + transpose + project piece h, accumulating into out_acc.
        def process_piece(h):
            nc.vector.tensor_copy(out=rden[:, h, :], in_=rsn[h][:, :, HD])
            # guard fully-masked rows (reference wipes them to 0): 0/eps -> 0
            nc.vector.tensor_scalar_max(rden[:, h, :], rden[:, h, :], 1e-30)
            nc.vector.reciprocal(rden[:, h, :], rden[:, h, :])
            snorms = []
            for b in range(2):
                snorm = small.tile([P, HD], BF16, tag="snorm", name=f"sn_{h}_{b}")
                nc.vector.tensor_scalar_mul(
                    snorm[:],
                    rsn[h][:, b, 0:HD],
                    rden[:, h, b : b + 1],
                )
                snorms.append(snorm)
            pst = mm_tile(f"st_{h}", BF16)
            for b in range(2):
                nc.tensor.transpose(
                    pst[:, 0, b * P : (b + 1) * P], snorms[b][:], ident
                )
            nc.vector.tensor_copy(out=sumT[:, h, :], in_=pst[:, 0, 0 : 2 * P])
            for b in range(2):
                po = mm_tile(f"po_{h}_{b}")
                for n in range(2):
                    nc.tensor.matmul(
                        po[:, n, :],
                        sumT[:, h, b * P : (b + 1) * P],
                        wf_sb[:, h, n * 512 : (n + 1) * 512],
                        start=True,
                        stop=True,
                    )
                if h == 0:
                    nc.vector.tensor_copy(out=out_acc[:, b], in_=po[:])
                else:
                    nc.vector.tensor_add(out_acc[:, b], out_acc[:, b], po[:])

        # --- attention pipeline: score head s+1 while accumulating head s;
        # fire a 2-rank RS after every head; pieces 0/1 are consumed two
        # heads later (hidden), pieces 2/3 in the short tail.
        for s in range(HL):
            avs = [av_tile(f"av_{s}_{qb}") for qb in range(QB)]
            prs = []
            for j in range(KC // 2):
                if s < HL - 1:
                    hn = s + 1
                    ps = mm_tile(f"ps_{hn}_{j}")
                    for half in range(2):
                        kc = j * 2 + half
                        nc.tensor.matmul(
                            ps[:, half, :],
                            kT[:, hn, kc * P : (kc + 1) * P],
                            qT[:, hn, :],
                            start=True,
                            stop=True,
                        )
                    pr = probs_pool.tile(
                        [P, 2, 512], BF16, tag="probs", name=f"pr_{hn}_{j}"
                    )
                    nc.scalar.activation(
                        pr[:], ps[:], mybir.ActivationFunctionType.Exp, scale=SCALE
                    )
                    nc.vector.tensor_mul(
                        pr[:], pr[:], mask_sb[:, j * 2 : j * 2 + 2, :]
                    )
                    prs.append(pr)
                # AV for head s, k-chunks 2j, 2j+1
                for half in range(2):
                    kc = j * 2 + half
                    for qb in range(QB):
                        if s == 0:
                            lhs = pp0[:, kc, qb * P : (qb + 1) * P]
                        else:
                            lhs = cur_prs[j][:, half, qb * P : (qb + 1) * P]
                        nc.tensor.matmul(
                            avs[qb][:],
                            lhs,
                            v_sb[:, kc, s, :],
                            start=(kc == 0),
                            stop=(kc == KC - 1),
                        )
            cur_prs = prs
            num = nums.tile([P, QB, HD + 1], BF16, tag="num", name=f"num_{s}")
            for qb in range(QB):
                nc.vector.tensor_copy(out=num[:, qb, :], in_=avs[qb][:])
            nc.sync.dma_start(
                out=rs_in[s].rearrange("a b p c -> p (a b) c"),
                in_=num[:],
            )
            rs_fire(s)
            # head h's RS (fired after head h) has ~2 heads of slack;
            # load + normalize + project it two heads later.
            if s >= 2:
                h = s - 2
                rsn_load(h)
                process_piece(h)

        for h in (HL - 2, HL - 1):
            rsn_load(h)
            process_piece(h)

        # --- store the [256, 1024] f32 partial (host sums head-halves).
        engs = [nc.sync, nc.scalar]
        for b in range(2):
            engs[b].dma_start(
                out=out[b * P : (b + 1) * P, :],
                in_=out_acc[:, b].rearrange("p n d -> p (n d)"),
            )

    nc.compile()
    return nc


_NC_CACHE = None


def _get_nc():
    global _NC_CACHE
    if _NC_CACHE is None:
        _NC_CACHE = build_attention_kernel()
    return _NC_CACHE


def make_in_maps(inputs):
    BF = ml_dtypes.bfloat16
    inputs = {k: np.asarray(v) for k, v in inputs.items()}
    WqT = np.asarray(inputs["Wq"]).T.astype(BF)  # [din, dout]
    WkT = np.asarray(inputs["Wk"]).T.astype(BF)
    WvT = np.asarray(inputs["Wv"]).T.astype(BF)
    WfT = np.asarray(inputs["Wf"]).T.astype(BF)  # [dsum, dout]
    xqTs = [
        np.ascontiguousarray(inputs["inputs_q"][b].T.astype(BF)) for b in range(B)
    ]
    in_maps = []
    for c in range(NCORES):
        b, hh, kh = c // 4, (c % 4) // 2, c % 2
        sl = slice(kh * KSH, (kh + 1) * KSH)
        hs = slice(hh * HL * HD, (hh + 1) * HL * HD)
        in_maps.append(
            {
                "xqT": xqTs[b],
                "xkT": np.ascontiguousarray(inputs["inputs_k"][b, sl].T.astype(BF)),
                "xvT": np.ascontiguousarray(inputs["inputs_v"][b, sl].T.astype(BF)),
                "mskT": np.ascontiguousarray(
                    inputs["attention_mask"][b, :, sl].T.astype(BF)
                ),
                "wqT": np.ascontiguousarray(WqT[:, hs]),
                "wkT": np.ascontiguousarray(WkT[:, hs]),
                "wvT": np.ascontiguousarray(WvT[:, hs]),
                "wfT": np.ascontiguousarray(WfT[hs, :]),
            }
        )
    return in_maps


def gather_out(results):
    out = np.zeros((B, QL, D), np.float32)
    for c in range(NCORES):
        b, kh = c // 4, c % 2
        r0 = kh * 256
        out[b, r0 : r0 + 256] += results[c]["out"]
    return out


def kernel(**inputs) -> np.ndarray:
    ensure_ntff_hook()  # defensive: BASS_TRACE=1 in env would need the shim
    from concourse.bass_utils import run_bass_kernel_spmd

    nc = _get_nc()
    in_maps = make_in_maps(inputs)
    res = run_bass_kernel_spmd(nc, in_maps, list(range(NCORES)))
    return gather_out(res.results)


# revision 19
# speedup vs baseline: 1.1076x; 1.0270x over previous
"""Multi-head attention kernel for Trainium2, SPMD over 8 NeuronCores.

Sharding: 2(batch) x 2(k-half) x 2(head-half). Each core holds one batch's
k/v slice of 4096 rows and computes K/V/Q projections + masked-softmax
attention for its 4 local heads. The only collectives are 2-rank
ReduceScatters (one ring step each, ~3x cheaper than the 4-rank variant)
that sum per-head attention numerators across the two k-halves; each core
then normalizes + projects its own q-half (256 rows) through its heads'
Wf rows, and the host sums the two head-half partials (same += gather the
4-way version used). No Q AllGather: each core projects Q for its own
heads locally.

Layout notes: all activations/weights/mask pre-transposed and pre-cast to
bf16 on the host; scores computed transposed ([k, q]) so the exp output is
directly the stationary operand of the AV matmul; multiplicative bf16 mask
after exp; softmax denominator rides as a 129th v-column through AV and
the ReduceScatter; no max-subtraction (scores are O(1)).

Engine schedule: scalar (ACT) runs only exp; vector owns all PSUM->SBUF
copies, mask multiplies, normalization and the f32 output accumulation.
Head 0's probs are precomputed during the V projection (ACT idle there);
the attention loop scores head s+1 while accumulating head s. Per-head RS
results are consumed two heads later (norm -> PE transpose -> per-head Wf
matmul -> f32 accum), so the PE never waits on a collective until the
final head's RS, whose post-work is ~3us. Startup DMAs are split into
512KB pieces across four queues (sync: xk stream, vector: wk, gpsimd: V
stream, scalar: wq/xq/wf/mask) so the first K-proj matmul starts ~1us in.
"""

import sys

if "/opt/trn_rl_repo" not in sys.path:
    sys.path.insert(0, "/opt/trn_rl_repo")

from contextlib import ExitStack

import ml_dtypes
import numpy as np

import concourse.bass as bass  # noqa: F401
import concourse.mybir as mybir
import concourse.tile as tile
from concourse import bacc
from concourse.masks import make_identity

B, QL, KL, D, H = 2, 512, 8192, 1024, 8
HD = D // H  # 128
NCORES = 8
PAIRS = [[0, 1], [2, 3], [4, 5], [6, 7]]  # k-half partners
KSH = KL // 2  # 4096 k rows per core
HL = 4  # local heads per core
SCALE = 1.0 / float(np.sqrt(HD))

F32 = mybir.dt.float32
BF16 = mybir.dt.bfloat16
P = 128
KC = KSH // P  # 32 k chunks of 128
QB = QL // P  # 4 q blocks
DB = D // P  # 8 d-in blocks
NCH = KSH // 512  # 8 streaming chunks of 512 k rows


def ensure_ntff_hook():
    """Provide antenv.axon_hooks (missing in this image) so trace=True works.

    Mirrors trn_agent_boot._ntff_profile_via_ctypes against the local
    libaxon_pjrt.so. No-op if the real module exists or the .so is absent.
    """
    try:
        import antenv.axon_hooks  # noqa: F401

        return
    except ImportError:
        pass
    import contextlib
    import ctypes
    import types

    mod = types.ModuleType("antenv.axon_hooks")
    holder = [None]
    mod.set_axon_ntff_profile_hook = lambda h: holder.__setitem__(0, h)
    mod.get_axon_ntff_profile_hook = lambda: holder[0]
    try:
        lib = ctypes.CDLL("/opt/axon/libaxon_pjrt.so")
        if hasattr(lib, "axon_start_nrt_profile"):
            lib.axon_start_nrt_profile.argtypes = [
                ctypes.POINTER(ctypes.c_int64),
                ctypes.c_size_t,
            ]
            lib.axon_start_nrt_profile.restype = ctypes.c_int64
            lib.axon_stop_nrt_profile.argtypes = [ctypes.c_char_p]
            lib.axon_stop_nrt_profile.restype = ctypes.c_int64

            @contextlib.contextmanager
            def _hook(output_dir, device_ids):
                import jax

                jax.devices()
                if device_ids:
                    ids = (ctypes.c_int64 * len(device_ids))(*device_ids)
                    rc = lib.axon_start_nrt_profile(ids, len(device_ids))
                else:
                    rc = lib.axon_start_nrt_profile(None, 0)
                if rc != 0:
                    raise RuntimeError(f"axon_start_nrt_profile rc={rc}")
                try:
                    yield
                finally:
                    n = lib.axon_stop_nrt_profile(str(output_dir).encode())
                    print(f"ntff profile: {n} file(s) -> {output_dir}")

            holder[0] = _hook
    except OSError:
        pass
    sys.modules["antenv.axon_hooks"] = mod
    try:
        import antenv

        antenv.axon_hooks = mod
    except ImportError:
        pass


def build_attention_kernel():
    nc = bacc.Bacc(
        "TRN2", target_bir_lowering=False, debug=False, num_devices=NCORES
    )

    xqT = nc.declare_dram_parameter("xqT", [D, QL], BF16, isOutput=False)
    xkT = nc.declare_dram_parameter("xkT", [D, KSH], BF16, isOutput=False)
    xvT = nc.declare_dram_parameter("xvT", [D, KSH], BF16, isOutput=False)
    mskT = nc.declare_dram_parameter("mskT", [KSH, QL], BF16, isOutput=False)
    wqT = nc.declare_dram_parameter("wqT", [D, HL * HD], BF16, isOutput=False)
    wkT = nc.declare_dram_parameter("wkT", [D, HL * HD], BF16, isOutput=False)
    wvT = nc.declare_dram_parameter("wvT", [D, HL * HD], BF16, isOutput=False)
    wfT = nc.declare_dram_parameter("wfT", [HL * HD, D], BF16, isOutput=False)
    out = nc.declare_dram_parameter("out", [2 * P, D], F32, isOutput=True)

    with tile.TileContext(nc) as tc, ExitStack() as ctx:
        # Persistent operand tiles (single-buffered, live for the kernel).
        persist = ctx.enter_context(tc.tile_pool(name="persist", bufs=1))
        kT = persist.tile([P, HL, KSH], BF16)  # [hd, head, krow]
        v_sb = persist.tile([P, KC, HL, HD + 1], BF16)  # [krow, kc, h, hd+1]
        mask_sb = persist.tile([P, KC, QL], BF16)  # [k, kc, q]
        qT = persist.tile([P, HL, QL], BF16)  # [hd, head, q]
        wv_sb = persist.tile([P, DB, HL * HD], BF16)
        wf_sb = persist.tile([P, HL, D], BF16)  # [hd, head, dout]
        sumT = persist.tile([P, HL, 2 * P], BF16)  # [hd, head, q-local]
        rsn = [
            persist.tile([P, 2, HD + 1], BF16, name=f"rsn{h}") for h in range(HL)
        ]
        rden = persist.tile([P, HL, 2], F32)
        out_acc = persist.tile([P, 2, 2, 512], F32)  # [q, qb-own, n, dout-half]

        # 32KB alias block: wk/wq/xq early, head-0 precomputed probs late.
        # (wk dies at K-proj end, wq/xq after the Q projection; pp0's first
        # write happens during the V projection, strictly later.)
        ablk = persist.tile([P, KC, QL], BF16, name="ablk")
        wk_sb = ablk[:, 0:DB, :]  # [P, 8, 512]
        wq_sb = ablk[:, DB : 2 * DB, :]
        xq_sb = ablk[:, 2 * DB : 3 * DB, :]
        pp0 = ablk  # [P, KC, QL] head-0 probs, precomputed

        loads = ctx.enter_context(tc.tile_pool(name="loads", bufs=2))
        probs_pool = ctx.enter_context(tc.tile_pool(name="probs", bufs=7))
        nums = ctx.enter_context(tc.tile_pool(name="nums", bufs=2))
        small = ctx.enter_context(tc.tile_pool(name="small", bufs=4))
        dram = ctx.enter_context(tc.tile_pool(name="dram", bufs=1, space="DRAM"))

        # rs_in[h]: [2 shares, 2 qb, 128, 129]; share r -> group rank r
        # (rank = k-half), i.e. q rows r*256..r*256+255 stay with rank r.
        rs_in = [
            dram.tile([2, 2, P, HD + 1], BF16, name=f"rs_in{h}")
            for h in range(HL)
        ]
        rs_out = [
            dram.tile([2, P, HD + 1], BF16, name=f"rs_out{h}") for h in range(HL)
        ]
        # tiny warmup collective: pays the first-call ncfw/CC staging cost
        # during the DMA-load phase instead of inside the attention loop.
        warm_in = dram.tile([2, P, 2], BF16, name="warm_in")
        warm_out = dram.tile([P, 2], BF16, name="warm_out")

        consts = ctx.enter_context(tc.tile_pool(name="consts", bufs=1))
        ident = consts.tile([P, P], BF16)
        make_identity(nc, ident)

        # One PSUM pool, 8 banks: mm 2x2 + av 4x1.
        psum = ctx.enter_context(tc.tile_pool(name="psum", bufs=1, space="PSUM"))

        def mm_tile(name, dtype=F32):
            return psum.tile([P, 2, 512], dtype, tag="mm", bufs=2, name=name)

        def av_tile(name, cols=HD + 1):
            return psum.tile([P, cols], F32, tag="av", bufs=4, name=name)

        # --- DMA loads, split into ~256-512KB pieces on three queues so the
        # first K-proj matmul can start ~1.5us in.
        #   sync:   xk stream (critical path, 2 DMAs per 512-chunk)
        #   scalar: wk first (parallel with xk), then wq, xq, wf
        #   gpsimd: mask, wv, then xv stream
        def load_pair(eng, dst, src, lo, hi):
            eng.dma_start(
                out=dst[:, lo // P : hi // P, :],
                in_=src[lo:hi, :].rearrange("(a p) d -> p a d", p=P),
            )

        def load_xchunk(eng, dst, src, c, half=None):
            lo, hi = (0, D) if half is None else (half * 512, (half + 1) * 512)
            eng.dma_start(
                out=dst[:, lo // P : hi // P, :],
                in_=src[lo:hi, c * 512 : (c + 1) * 512].rearrange(
                    "(a p) k -> p a k", p=P
                ),
            )

        for i in range(4):
            load_pair(nc.scalar, wk_sb, wkT, i * 256, (i + 1) * 256)
        xkc0 = loads.tile([P, DB, 512], BF16, tag="ld", name="xkc0")
        for half in range(2):
            load_xchunk(nc.sync, xkc0, xkT, 0, half)
        xkc1 = loads.tile([P, DB, 512], BF16, tag="ld", name="xkc1")
        for half in range(2):
            load_xchunk(nc.sync, xkc1, xkT, 1, half)

        load_pair(nc.scalar, wq_sb, wqT, 0, 512)
        load_pair(nc.scalar, wq_sb, wqT, 512, 1024)
        load_pair(nc.scalar, xq_sb, xqT, 0, 512)
        load_pair(nc.scalar, xq_sb, xqT, 512, 1024)
        nc.scalar.dma_start(
            out=wf_sb, in_=wfT.rearrange("(i p) d -> p i d", p=P)
        )

        for i in range(4):
            nc.gpsimd.dma_start(
                out=mask_sb[:, i * 8 : (i + 1) * 8, :],
                in_=mskT[i * 1024 : (i + 1) * 1024, :].rearrange(
                    "(a p) q -> p a q", p=P
                ),
            )
        load_pair(nc.gpsimd, wv_sb, wvT, 0, 512)
        load_pair(nc.gpsimd, wv_sb, wvT, 512, 1024)
        # warmup collective staging DMA (collective fired after the V DMA
        # stream is fully issued; the trigger blocks the gpsimd engine).
        nc.vector.memset(sumT[:, 0:2, 0:2], 0.0)
        nc.sync.dma_start(
            out=warm_in.rearrange("s p c -> p s c"), in_=sumT[:, 0:2, 0:2]
        )

        # --- K projection: 8 chunks of 512 k rows; 2 head-pairs each.
        def k_proj_chunk(c, xkc):
            for hp in range(2):
                pk = mm_tile(f"pk_{c}_{hp}")
                for i in range(2):
                    for a in range(DB):
                        nc.tensor.matmul(
                            pk[:, i, :],
                            wk_sb[:, a, hp * 256 + i * HD : hp * 256 + (i + 1) * HD],
                            xkc[:, a, :],
                            start=(a == 0),
                            stop=(a == DB - 1),
                        )
                nc.vector.tensor_copy(
                    out=kT[:, 2 * hp : 2 * hp + 2, c * 512 : (c + 1) * 512],
                    in_=pk[:],
                )

        k_proj_chunk(0, xkc0)
        xkc2 = loads.tile([P, DB, 512], BF16, tag="ld", name="xkc2")
        for half in range(2):
            load_xchunk(nc.sync, xkc2, xkT, 2, half)
        k_proj_chunk(1, xkc1)
        xkc3 = loads.tile([P, DB, 512], BF16, tag="ld", name="xkc3")
        for half in range(2):
            load_xchunk(nc.sync, xkc3, xkT, 3, half)
        k_proj_chunk(2, xkc2)

        # --- Q projection for this core's 4 heads (local; no AllGather).
        # Placed here so its wq/xq DMAs (behind wk on scalar) have landed.
        for hp in range(2):
            pq = mm_tile(f"pq_{hp}")
            for i in range(2):
                for a in range(DB):
                    nc.tensor.matmul(
                        pq[:, i, :],
                        wq_sb[:, a, hp * 256 + i * HD : hp * 256 + (i + 1) * HD],
                        xq_sb[:, a, :],
                        start=(a == 0),
                        stop=(a == DB - 1),
                    )
            nc.vector.tensor_copy(out=qT[:, 2 * hp : 2 * hp + 2, :], in_=pq[:])

        nxt = xkc3
        for c in range(3, NCH):
            if c < NCH - 1:
                nxtc = loads.tile([P, DB, 512], BF16, tag="ld", name=f"xkc{c + 1}")
                for half in range(2):
                    load_xchunk(nc.sync, nxtc, xkT, c + 1, half)
            k_proj_chunk(c, nxt)
            nxt = nxtc if c < NCH - 1 else None

        # --- V projection (xvT streamed); head-0 probs precomputed alongside
        # (ACT is otherwise idle here). One pre_probs per (c, mkl).
        def pre_probs(kc):
            ps = av_tile(f"pps_{kc}", 512)
            nc.tensor.matmul(
                ps[:],
                kT[:, 0, kc * P : (kc + 1) * P],
                qT[:, 0, :],
                start=True,
                stop=True,
            )
            nc.scalar.activation(
                pp0[:, kc, :], ps[:], mybir.ActivationFunctionType.Exp, scale=SCALE
            )
            nc.vector.tensor_mul(
                pp0[:, kc, :], pp0[:, kc, :], mask_sb[:, kc, :]
            )

        xvc_next = loads.tile([P, DB, 512], BF16, tag="ld", name="xvc0")
        for half in range(2):
            load_xchunk(nc.gpsimd, xvc_next, xvT, 0, half)
        for c in range(NCH):
            xvc = xvc_next
            if c < NCH - 1:
                xvc_next = loads.tile(
                    [P, DB, 512], BF16, tag="ld", name=f"xvc{c + 1}"
                )
                for half in range(2):
                    load_xchunk(nc.gpsimd, xvc_next, xvT, c + 1, half)
            if c == NCH - 1:
                # warmup collective (result unused): absorbs the first-call
                # CC staging latency while the PE finishes the V projection.
                nc.gpsimd.collective_compute(
                    "ReduceScatter",
                    mybir.AluOpType.add,
                    replica_groups=PAIRS,
                    ins=[warm_in.opt()],
                    outs=[warm_out.opt()],
                )
            for mkl in range(4):
                mk = c * 4 + mkl
                pv = av_tile(f"pv_{mk}", 512)
                for a in range(DB):
                    nc.tensor.matmul(
                        pv[:],
                        xvc[:, a, mkl * P : (mkl + 1) * P],
                        wv_sb[:, a, :],
                        start=(a == 0),
                        stop=(a == DB - 1),
                    )
                nc.vector.tensor_copy(
                    out=v_sb[:, mk, :, 0:HD],
                    in_=pv[:].rearrange("p (b c) -> p b c", b=HL),
                )
                pre_probs(mk)
        nc.vector.memset(v_sb[:, :, :, HD], 1.0)

        # --- per-head numerator ReduceScatter (2-rank, k-half partners).
        def rs_fire(h):
            nc.gpsimd.collective_compute(
                "ReduceScatter",
                mybir.AluOpType.add,
                replica_groups=PAIRS,
                ins=[rs_in[h].opt()],
                outs=[rs_out[h].opt()],
            )

        def rsn_load(h):
            nc.sync.dma_start(
                out=rsn[h][:], in_=rs_out[h].rearrange("b p c -> p b c")
            )

        # normalize + transpose + project piece h, accumulating into out_acc.
        def process_piece(h):
            nc.vector.tensor_copy(out=rden[:, h, :], in_=rsn[h][:, :, HD])
            # guard fully-masked rows (reference wipes them to 0): 0/eps -> 0
            nc.vector.tensor_scalar_max(rden[:, h, :], rden[:, h, :], 1e-30)
            nc.vector.reciprocal(rden[:, h, :], rden[:, h, :])
            snorms = []
            for b in range(2):
                snorm = small.tile([P, HD], BF16, tag="snorm", name=f"sn_{h}_{b}")
                nc.vector.tensor_scalar_mul(
                    snorm[:],
                    rsn[h][:, b, 0:HD],
                    rden[:, h, b : b + 1],
                )
                snorms.append(snorm)
            pst = mm_tile(f"st_{h}", BF16)
            for b in range(2):
                nc.tensor.transpose(
                    pst[:, 0, b * P : (b + 1) * P], snorms[b][:], ident
                )
            nc.vector.tensor_copy(out=sumT[:, h, :], in_=pst[:, 0, 0 : 2 * P])
            for b in range(2):
                po = mm_tile(f"po_{h}_{b}")
                for n in range(2):
                    nc.tensor.matmul(
                        po[:, n, :],
                        sumT[:, h, b * P : (b + 1) * P],
                        wf_sb[:, h, n * 512 : (n + 1) * 512],
                        start=True,
                        stop=True,
                    )
                if h == 0:
                    nc.vector.tensor_copy(out=out_acc[:, b], in_=po[:])
                else:
                    nc.vector.tensor_add(out_acc[:, b], out_acc[:, b], po[:])

        # --- attention pipeline: score head s+1 while accumulating head s;
        # fire a 2-rank RS after every head; pieces 0/1 are consumed two
        # heads later (hidden), pieces 2/3 in the short tail.
        for s in range(HL):
            avs = [av_tile(f"av_{s}_{qb}") for qb in range(QB)]
            prs = []
            for j in range(KC // 2):
                if s < HL - 1:
                    hn = s + 1
                    ps = mm_tile(f"ps_{hn}_{j}")
                    for half in range(2):
                        kc = j * 2 + half
                        nc.tensor.matmul(
                            ps[:, half, :],
                            kT[:, hn, kc * P : (kc + 1) * P],
                            qT[:, hn, :],
                            start=True,
                            stop=True,
                        )
                    pr = probs_pool.tile(
                        [P, 2, 512], BF16, tag="probs", name=f"pr_{hn}_{j}"
                    )
                    nc.scalar.activation(
                        pr[:], ps[:], mybir.ActivationFunctionType.Exp, scale=SCALE
                    )
                    nc.vector.tensor_mul(
                        pr[:], pr[:], mask_sb[:, j * 2 : j * 2 + 2, :]
                    )
                    prs.append(pr)
                # AV for head s, k-chunks 2j, 2j+1
                for half in range(2):
                    kc = j * 2 + half
                    for qb in range(QB):
                        if s == 0:
                            lhs = pp0[:, kc, qb * P : (qb + 1) * P]
                        else:
                            lhs = cur_prs[j][:, half, qb * P : (qb + 1) * P]
                        nc.tensor.matmul(
                            avs[qb][:],
                            lhs,
                            v_sb[:, kc, s, :],
                            start=(kc == 0),
                            stop=(kc == KC - 1),
                        )
            cur_prs = prs
            num = nums.tile([P, QB, HD + 1], BF16, tag="num", name=f"num_{s}")
            for qb in range(QB):
                nc.vector.tensor_copy(out=num[:, qb, :], in_=avs[qb][:])
            nc.sync.dma_start(
                out=rs_in[s].rearrange("a b p c -> p (a b) c"),
                in_=num[:],
            )
            rs_fire(s)
            # head h's RS (fired after head h) has ~2 heads of slack;
            # load + normalize + project it two heads later.
            if s >= 2:
                h = s - 2
                rsn_load(h)
                process_piece(h)

        for h in (HL - 2, HL - 1):
            rsn_load(h)
            process_piece(h)

        # --- store the [256, 1024] f32 partial (host sums head-halves).
        engs = [nc.sync, nc.scalar]
        for b in range(2):
            engs[b].dma_start(
                out=out[b * P : (b + 1) * P, :],
                in_=out_acc[:, b].rearrange("p n d -> p (n d)"),
            )

    nc.compile()
    return nc


_NC_CACHE = None


def _get_nc():
    global _NC_CACHE
    if _NC_CACHE is None:
        _NC_CACHE = build_attention_kernel()
    return _NC_CACHE


def make_in_maps(inputs):
    BF = ml_dtypes.bfloat16
    inputs = {k: np.asarray(v) for k, v in inputs.items()}
    WqT = np.asarray(inputs["Wq"]).T.astype(BF)  # [din, dout]
    WkT = np.asarray(inputs["Wk"]).T.astype(BF)
    WvT = np.asarray(inputs["Wv"]).T.astype(BF)
    WfT = np.asarray(inputs["Wf"]).T.astype(BF)  # [dsum, dout]
    xqTs = [
        np.ascontiguousarray(inputs["inputs_q"][b].T.astype(BF)) for b in range(B)
    ]
    in_maps = []
    for c in range(NCORES):
        b, hh, kh = c // 4, (c % 4) // 2, c % 2
        sl = slice(kh * KSH, (kh + 1) * KSH)
        hs = slice(hh * HL * HD, (hh + 1) * HL * HD)
        in_maps.append(
            {
                "xqT": xqTs[b],
                "xkT": np.ascontiguousarray(inputs["inputs_k"][b, sl].T.astype(BF)),
                "xvT": np.ascontiguousarray(inputs["inputs_v"][b, sl].T.astype(BF)),
                "mskT": np.ascontiguousarray(
                    inputs["attention_mask"][b, :, sl].T.astype(BF)
                ),
                "wqT": np.ascontiguousarray(WqT[:, hs]),
                "wkT": np.ascontiguousarray(WkT[:, hs]),
                "wvT": np.ascontiguousarray(WvT[:, hs]),
                "wfT": np.ascontiguousarray(WfT[hs, :]),
            }
        )
    return in_maps


def gather_out(results):
    out = np.zeros((B, QL, D), np.float32)
    for c in range(NCORES):
        b, kh = c // 4, c % 2
        r0 = kh * 256
        out[b, r0 : r0 + 256] += results[c]["out"]
    return out


def kernel(**inputs) -> np.ndarray:
    ensure_ntff_hook()  # defensive: BASS_TRACE=1 in env would need the shim
    from concourse.bass_utils import run_bass_kernel_spmd

    nc = _get_nc()
    in_maps = make_in_maps(inputs)
    res = run_bass_kernel_spmd(nc, in_maps, list(range(NCORES)))
    return gather_out(res.results)
